# revision 9
# baseline (speedup 1.0000x reference)
"""Trainium2 Bass kernel v3 for sparse 3D conv (gather -> 8x[32,32] GEMM -> scatter-add).

Tunnel-transfer-optimized vs v1:
- x shipped pre-transposed (lhsT layout) in int8 with the global scale folded
  into the bf16 weights -> no on-device transpose, 4x fewer x bytes; the DVE
  upconverts int8 -> bf16 (exact) before the GEMMs.
- static slot->k schedule: slot s (128 tokens) of every 1024-token scatter
  call applies W[s] -- no streamed per-block weight tensor at all.
- scatter indices shipped once [16, cols] and replicated to 128 partitions
  on device (8x fewer idx bytes).
- scatter-add goes to an Internal f32 scratch (row stride 256B as HW
  requires), then a compaction pass rescales each output row by its absmax
  and emits int8 rows + fp16 per-row scales (~7.5x fewer output bytes than
  the v1 f32 stride-64 layout; the donated-zero H2D leg shrinks equally).
  Row error <= amax/127 -> ~8e-3 of the global max worst case.

Scatter-race safety (the in-flight window can span calls; there is NO
DMA-completion serialization between scatter calls -- verified on emitted
BIR sync_info):
- duplicate (k, out) points are level-decomposed: level-0 tokens go to the
  phase-scheduled main calls (so every token of a (band, k) main stream
  targets a distinct row); levels >= 1 go to trailing per-band spill calls,
  greedily binned so no spill call holds two tokens of the same row (a row's
  j-th duplicate lands in spill call >= j);
- per (band, k) the out-sorted level-0 tokens are chopped into 128-token
  granules; granule g runs at cycle (g - phase_k) mod C with
  phase_k = k*C//8, C >= 24. Same-row tokens of different k then differ by
  >= 1 cycle physically (circular phase gap >= 3 vs <= 2 granules of cross-k
  rank noise), so they are never in the same 1024-token call, and with the
  band-interleaved queue order (queue q carries bands q and q+4 alternately)
  their queue-stream separation is >= ~1793 tokens -- 3.5x the v1 spill
  guard;
- pad tokens all target a discarded dummy row (concurrent RMW there is
  harmless).
"""

import sys

sys.path.insert(0, "/opt/trn_rl_repo")

import numpy as np
import ml_dtypes

import concourse.bacc as bacc
import concourse.mybir as mybir
import concourse.tile as tile

P = 128
TOK = 1024  # tokens per scatter call: 8 slots x 128
N_CORES = 8
N_BANDS = 8  # per core
NG = N_CORES * N_BANDS
N_Q = 4
C_MIN = 24  # keeps the circular phase gap >= 3 cycles

X_NP = np.int8
X_BIR = mybir.dt.int8


def host_prepare(x, weight, offset_idx, out_idx):
    N = x.shape[0]
    M = int(out_idx.max()) + 1
    r_band = -(-M // NG)
    r_pad = -(-(r_band + 1) // P) * P  # scratch/output rows per band

    x = np.asarray(x, np.float32)
    xscale = float(np.abs(x).max()) / 127.0

    band = out_idx // r_band
    rowin_all = (out_idx - band * r_band).astype(np.int16)

    # duplicate level of each point within its (band->k->out) group
    ord0 = np.lexsort((out_idx, offset_idx, band))
    b_o, k_o, out_o = band[ord0], offset_idx[ord0], out_idx[ord0]
    new_run = np.ones(N, bool)
    new_run[1:] = (b_o[1:] != b_o[:-1]) | (k_o[1:] != k_o[:-1]) | (out_o[1:] != out_o[:-1])
    rstarts = np.flatnonzero(new_run)
    lvl_sorted = np.arange(N) - np.repeat(rstarts, np.diff(np.append(rstarts, N)))
    lvl = np.empty(N, np.int64)
    lvl[ord0] = lvl_sorted

    # ---- main stream: level-0 tokens, per (band, k) sorted by out ----
    main_ids = ord0[lvl_sorted == 0]  # already (band, k, out)-sorted
    gk = band[main_ids] * 8 + offset_idx[main_ids]
    U = main_ids.size
    gstarts = np.flatnonzero(np.diff(np.append(-1, gk)) != 0)
    gcounts = np.diff(np.append(gstarts, U))
    rank = np.arange(U) - np.repeat(gstarts, gcounts)
    cnt = np.bincount(gk, minlength=NG * 8)
    C = max(C_MIN, int(-(-cnt.max() // P)))

    km = offset_idx[main_ids]
    g = rank // P
    gi_m = rank % P
    ph = (km * C) // 8
    cyc_m = (g - ph) % C

    # ---- spill tokens (lvl >= 1): per band, row-unique greedy call binning ----
    spill_ids = ord0[lvl_sorted >= 1]  # sorted by (band, k, out, lvl)
    sb = band[spill_ids]
    sk = offset_idx[spill_ids]
    sout = out_idx[spill_ids]
    # order per band by (out, lvl-ish): resort by (band, out, k)
    ords = np.lexsort((lvl[spill_ids], sk, sout, sb))
    spill_ids = spill_ids[ords]
    sb, sk, sout = sb[ords], sk[ords], sout[ords]
    cyc_s = np.empty(spill_ids.size, np.int64)
    gi_s = np.empty(spill_ids.size, np.int64)
    S = 0
    if spill_ids.size:
        cap = {}
        prev_row = None
        jprev = -1
        prev_b = -1
        for i in range(spill_ids.size):
            bq, kq, rq = sb[i], sk[i], sout[i]
            if bq != prev_b:
                cap = {}
                prev_b = bq
                prev_row = None
            if rq != prev_row:
                jprev = -1
                prev_row = rq
            j = jprev + 1
            while cap.get((j, kq), 0) >= P:
                j += 1
            gi_s[i] = cap.get((j, kq), 0)
            cap[(j, kq)] = gi_s[i] + 1
            cyc_s[i] = C + j
            jprev = j
            S = max(S, j + 1)

    CT = C + S  # calls per band
    n_calls = N_BANDS * CT

    ids = np.concatenate([main_ids, spill_ids])
    kk = np.concatenate([km, sk])
    cyc = np.concatenate([cyc_m, cyc_s])
    gi = np.concatenate([gi_m, gi_s])

    s = kk  # slot = k
    fg = s // 4
    lr = s % 4
    bb = band[ids]
    core = bb // N_BANDS
    e = bb % N_BANDS
    call_prog = cyc * N_BANDS + e
    col = call_prog * 256 + fg * P + gi
    p_in_call = s * P + gi

    xq = np.round(x / xscale).astype(np.int8)
    XT = np.zeros((N_CORES, P, n_calls * 256), X_NP)
    for lrv in range(4):
        m = lr == lrv
        XT[:, 32 * lrv : 32 * lrv + 32, :][core[m], :, col[m]] = xq[ids[m]]

    icols = TOK // 16
    IDX = np.full((N_CORES, 16, n_calls * icols), r_band, np.int16)
    IDX[core, p_in_call % 16, call_prog * icols + p_in_call // 16] = rowin_all[ids]

    w = (np.asarray(weight, np.float32) * xscale).astype(ml_dtypes.bfloat16)
    wall = np.zeros((P, 64), ml_dtypes.bfloat16)
    for k in range(8):
        wall[32 * (k % 4) : 32 * (k % 4) + 32, 32 * (k // 4) : 32 * (k // 4) + 32] = w[k]

    cores = [{"xsT": XT[cc], "idx": IDX[cc], "wall": wall} for cc in range(N_CORES)]
    meta = {"r_band": r_band, "r_pad": r_pad, "C": CT, "M": M}
    return cores, meta


def build_bass(meta):
    r_band = meta["r_band"]
    r_pad = meta["r_pad"]
    C = meta["C"]
    n_calls = N_BANDS * C
    FR = r_pad // P  # compaction frames per band
    icols = TOK // 16

    nc = bacc.Bacc("TRN2", num_swdge_queues=N_Q)
    xsT = nc.dram_tensor("xsT", [P, n_calls * 256], X_BIR, kind="ExternalInput")
    idx = nc.dram_tensor("idx", [16, n_calls * icols], mybir.dt.int16, kind="ExternalInput")
    wall = nc.dram_tensor("wall", [P, 64], mybir.dt.bfloat16, kind="ExternalInput")
    yq = nc.dram_tensor("yq", [N_BANDS * r_pad, 32], mybir.dt.int8, kind="ExternalOutput")
    ys = nc.dram_tensor("ys", [N_BANDS * r_pad], mybir.dt.float16, kind="ExternalOutput")
    scr = [
        nc.dram_tensor(f"scr_{q}", [2 * r_pad, 64], mybir.dt.float32, kind="Internal")
        for q in range(N_Q)
    ]
    # block-contiguous per-band views: partition p owns rows [p*FR, (p+1)*FR)
    # of the band -> every DMA span below is contiguous per partition (1
    # descriptor/partition instead of 1 per 128B frame; keeps the NEFF small)
    yqv = [
        yq[e * r_pad : (e + 1) * r_pad, :].rearrange("(p n) c -> p n c", p=P)
        for e in range(N_BANDS)
    ]
    ysv = [
        ys[e * r_pad : (e + 1) * r_pad].rearrange("(p n) -> p n", p=P)
        for e in range(N_BANDS)
    ]

    with tile.TileContext(nc) as tc:
        with (
            tc.tile_pool(name="wp", bufs=1) as wpool,
            tc.tile_pool(name="ip", bufs=1) as ipool,
            tc.tile_pool(name="zp", bufs=1) as zpool,
            tc.tile_pool(name="xp", bufs=4) as xpool,
            tc.tile_pool(name="xc", bufs=4) as xcpool,
            tc.tile_pool(name="st", bufs=8) as stpool,
            tc.tile_pool(name="pz", bufs=8, space="PSUM") as pzpool,
            tc.tile_pool(name="cp", bufs=4) as cpool,
            tc.tile_pool(name="sc", bufs=4) as scpool,
            tc.tile_pool(name="qp", bufs=4) as qpool,
        ):
            wt = wpool.tile([P, 64], mybir.dt.bfloat16, tag="w")
            nc.sync.dma_start(out=wt[:], in_=wall[:, :])
            it = ipool.tile([P, n_calls * icols], mybir.dt.int16, tag="idx")
            for gpart in range(8):
                nc.sync.dma_start(
                    out=it[16 * gpart : 16 * gpart + 16, :], in_=idx[:, :]
                )
            # zero the scatter scratch (block-contiguous per-half views)
            zt = zpool.tile([P, 2048], mybir.dt.float32, tag="z")
            nc.vector.memset(zt[:], 0.0)
            for q in range(N_Q):
                for half in range(2):
                    svh = scr[q][half * r_pad : (half + 1) * r_pad, :].rearrange(
                        "(p n) c -> p n c", p=P
                    )  # [128, FR, 64]
                    f0 = 0
                    while f0 < FR:
                        fcnt = min(32, FR - f0)
                        nc.sync.dma_start(
                            out=svh[:, f0 : f0 + fcnt, :],
                            in_=zt[:, : fcnt * 64].rearrange("p (n c) -> p n c", c=64),
                        )
                        f0 += fcnt

            for c in range(C):
                for e in range(N_BANDS):
                    call_prog = c * N_BANDS + e
                    xb8 = xpool.tile([P, 256], X_BIR, tag="x")
                    nc.sync.dma_start(
                        out=xb8[:], in_=xsT[:, call_prog * 256 : (call_prog + 1) * 256]
                    )
                    xb = xcpool.tile([P, 256], mybir.dt.bfloat16, tag="xc")
                    nc.vector.tensor_copy(out=xb[:], in_=xb8[:])
                    st = stpool.tile([P, 8, 32], mybir.dt.float32, tag="st")
                    for fg in range(2):
                        for lrv in range(4):
                            pz = pzpool.tile([P, 32], mybir.dt.float32, tag="pz")
                            nc.tensor.matmul(
                                out=pz[:],
                                lhsT=xb[32 * lrv : 32 * lrv + 32, P * fg : P * (fg + 1)],
                                rhs=wt[32 * lrv : 32 * lrv + 32, 32 * fg : 32 * fg + 32],
                                start=True,
                                stop=True,
                                tile_position=(32 * lrv, 0),
                            )
                            nc.vector.tensor_copy(out=st[:, 4 * fg + lrv, :], in_=pz[:])
                    q = e % N_Q
                    off = 0 if e < N_Q else r_pad
                    nc.gpsimd.dma_scatter_add(
                        scr[q][off : off + r_band + 1, :32],
                        st[:],
                        it[:, call_prog * icols : (call_prog + 1) * icols],
                        TOK,
                        TOK,
                        32,
                        elem_step=64,
                        queue_num=q,
                    )

            # compact scratch f32 [*, 64] -> int8 rows + fp16 row scales.
            # reads pull the full 64-col row (contiguous per partition); the
            # DVE ops then address only cols :32 via strided APs.
            CH = 16  # frames per chunk
            for e in range(N_BANDS):
                q = e % N_Q
                half = e // N_Q
                svh = scr[q][half * r_pad : (half + 1) * r_pad, :].rearrange(
                    "(p n) c -> p n c", p=P
                )
                f0 = 0
                while f0 < FR:
                    fcnt = min(CH, FR - f0)
                    ct = cpool.tile([P, CH, 64], mybir.dt.float32, tag="ct")
                    nc.sync.dma_start(
                        out=ct[:, :fcnt, :],
                        in_=svh[:, f0 : f0 + fcnt, :],
                    )
                    amax = scpool.tile([P, CH], mybir.dt.float32, tag="amax")
                    nc.vector.tensor_reduce(
                        out=amax[:, :fcnt],
                        in_=ct[:, :fcnt, :32],
                        axis=mybir.AxisListType.X,
                        op=mybir.AluOpType.max,
                        apply_absolute_value=True,
                    )
                    nc.vector.tensor_scalar_max(
                        out=amax[:, :fcnt], in0=amax[:, :fcnt], scalar1=1e-30
                    )
                    sca = scpool.tile([P, CH], mybir.dt.float32, tag="sca")
                    nc.vector.reciprocal(out=sca[:, :fcnt], in_=amax[:, :fcnt])
                    nc.vector.tensor_scalar_mul(
                        out=sca[:, :fcnt], in0=sca[:, :fcnt], scalar1=127.0
                    )
                    smx = scpool.tile([P, CH], mybir.dt.float16, tag="smx")
                    nc.vector.tensor_copy(out=smx[:, :fcnt], in_=amax[:, :fcnt])
                    nc.sync.dma_start(
                        out=ysv[e][:, f0 : f0 + fcnt],
                        in_=smx[:, :fcnt],
                    )
                    nc.vector.tensor_mul(
                        out=ct[:, :fcnt, :32],
                        in0=ct[:, :fcnt, :32],
                        in1=sca[:, :fcnt].unsqueeze(-1).broadcast_to((P, fcnt, 32)),
                    )
                    qt = qpool.tile([P, CH, 32], mybir.dt.int8, tag="qt")
                    nc.vector.tensor_copy(out=qt[:, :fcnt, :], in_=ct[:, :fcnt, :32])
                    nc.sync.dma_start(
                        out=yqv[e][:, f0 : f0 + fcnt, :],
                        in_=qt[:, :fcnt, :],
                    )
                    f0 += fcnt
    nc.compile()
    return nc


def kernel(x, weight, offset_idx, out_idx, num_out):
    from concourse.bass_utils import run_bass_kernel_spmd

    x = np.asarray(x, np.float32)
    weight = np.asarray(weight, np.float32)
    offset_idx = np.asarray(offset_idx, np.int64)
    out_idx = np.asarray(out_idx, np.int64)
    num_out = int(num_out)

    cores, meta = host_prepare(x, weight, offset_idx, out_idx)
    nc = build_bass(meta)
    in_maps = [{"xsT": c["xsT"], "idx": c["idx"], "wall": c["wall"]} for c in cores]
    res = run_bass_kernel_spmd(nc, in_maps, core_ids=list(range(N_CORES)))

    r_band = meta["r_band"]
    r_pad = meta["r_pad"]
    M = meta["M"]
    y = np.zeros((num_out, 32), np.float32)
    for cc in range(N_CORES):
        yqc = res.results[cc]["yq"].reshape(N_BANDS, r_pad, 32)
        ysc = res.results[cc]["ys"].reshape(N_BANDS, r_pad)
        for e in range(N_BANDS):
            gb = cc * N_BANDS + e
            r0 = gb * r_band
            r1 = min(r0 + r_band, M)
            if r1 <= r0:
                continue
            n = r1 - r0
            y[r0:r1] = yqc[e, :n].astype(np.float32) * (
                ysc[e, :n, None].astype(np.float32) / 127.0
            )
    return y


# revision 10
# speedup vs baseline: 1.0755x; 1.0755x over previous
"""Trainium2 Bass kernel v3 for sparse 3D conv (gather -> 8x[32,32] GEMM -> scatter-add).

Tunnel-transfer-optimized vs v1:
- x shipped pre-transposed (lhsT layout) in int8 with the global scale folded
  into the bf16 weights -> no on-device transpose, 4x fewer x bytes; the DVE
  upconverts int8 -> bf16 (exact) before the GEMMs.
- static slot->k schedule: slot s (128 tokens) of every 1024-token scatter
  call applies W[s] -- no streamed per-block weight tensor at all.
- scatter indices shipped once [16, cols] and replicated to 128 partitions
  on device (8x fewer idx bytes).
- scatter-add goes to an Internal f32 scratch (row stride 256B as HW
  requires), then a compaction pass rescales each output row by its absmax
  and emits int8 rows + fp16 per-row scales (~7.5x fewer output bytes than
  the v1 f32 stride-64 layout; the donated-zero H2D leg shrinks equally).
  Row error <= amax/127 -> ~8e-3 of the global max worst case.

Scatter-race safety (the in-flight window can span calls; there is NO
DMA-completion serialization between scatter calls -- verified on emitted
BIR sync_info):
- duplicate (k, out) points are level-decomposed: level-0 tokens go to the
  phase-scheduled main calls (so every token of a (band, k) main stream
  targets a distinct row); levels >= 1 go to trailing per-band spill calls,
  greedily binned so no spill call holds two tokens of the same row (a row's
  j-th duplicate lands in spill call >= j);
- per (band, k) the out-sorted level-0 tokens are chopped into 128-token
  granules; granule g runs at cycle (g - phase_k) mod C with
  phase_k = k*C//8, C >= 24. Same-row tokens of different k then differ by
  >= 1 cycle physically (circular phase gap >= 3 vs <= 2 granules of cross-k
  rank noise), so they are never in the same 1024-token call, and with the
  band-interleaved queue order (queue q carries bands q and q+4 alternately)
  their queue-stream separation is >= ~1793 tokens -- 3.5x the v1 spill
  guard;
- pad tokens all target a discarded dummy row (concurrent RMW there is
  harmless).
"""

import sys

sys.path.insert(0, "/opt/trn_rl_repo")

import numpy as np
import ml_dtypes

import concourse.bacc as bacc
import concourse.mybir as mybir
import concourse.tile as tile

P = 128
TOK = 1024  # tokens per scatter call: 8 slots x 128
N_CORES = 8
N_BANDS = 8  # per core
NG = N_CORES * N_BANDS
N_Q = 4
C_MIN = 24  # keeps the circular phase gap >= 3 cycles

X_NP = np.int8
X_BIR = mybir.dt.int8


def host_prepare(x, weight, offset_idx, out_idx):
    N = x.shape[0]
    M = int(out_idx.max()) + 1
    r_band = -(-M // NG)
    r_pad = -(-(r_band + 1) // P) * P  # scratch/output rows per band

    x = np.asarray(x, np.float32)
    xscale = float(np.abs(x).max()) / 127.0

    band = out_idx // r_band
    rowin_all = (out_idx - band * r_band).astype(np.int16)

    # duplicate level of each point within its (band->k->out) group
    ord0 = np.lexsort((out_idx, offset_idx, band))
    b_o, k_o, out_o = band[ord0], offset_idx[ord0], out_idx[ord0]
    new_run = np.ones(N, bool)
    new_run[1:] = (b_o[1:] != b_o[:-1]) | (k_o[1:] != k_o[:-1]) | (out_o[1:] != out_o[:-1])
    rstarts = np.flatnonzero(new_run)
    lvl_sorted = np.arange(N) - np.repeat(rstarts, np.diff(np.append(rstarts, N)))
    lvl = np.empty(N, np.int64)
    lvl[ord0] = lvl_sorted

    # ---- main stream: level-0 tokens, per (band, k) sorted by out ----
    main_ids = ord0[lvl_sorted == 0]  # already (band, k, out)-sorted
    gk = band[main_ids] * 8 + offset_idx[main_ids]
    U = main_ids.size
    gstarts = np.flatnonzero(np.diff(np.append(-1, gk)) != 0)
    gcounts = np.diff(np.append(gstarts, U))
    rank = np.arange(U) - np.repeat(gstarts, gcounts)
    cnt = np.bincount(gk, minlength=NG * 8)
    C = max(C_MIN, int(-(-cnt.max() // P)))

    km = offset_idx[main_ids]
    g = rank // P
    gi_m = rank % P
    ph = (km * C) // 8
    cyc_m = (g - ph) % C

    # ---- spill tokens (lvl >= 1): overlay into spare slots of main calls,
    # keeping <= 1 token per output row per call (the race invariant).
    # Per (band, k, cycle) the slot fill from main tokens is known; a spill
    # for row R goes to any cycle with slot space where R does not already
    # appear. Rare overflow falls back to trailing dedicated calls.
    spill_ids = ord0[lvl_sorted >= 1]  # sorted by (band, k, out, lvl)
    sb = band[spill_ids]
    sk = offset_idx[spill_ids]
    sout = out_idx[spill_ids]
    ords = np.lexsort((lvl[spill_ids], sk, sout, sb))
    spill_ids = spill_ids[ords]
    sb, sk, sout = sb[ords], sk[ords], sout[ords]
    cyc_s = np.empty(spill_ids.size, np.int64)
    gi_s = np.empty(spill_ids.size, np.int64)
    S = 0
    if spill_ids.size:
        # slot fill per (band, k, cycle) from the main assignment
        fill = np.zeros((NG, 8, C + 8), np.int64)
        np.add.at(fill, (band[main_ids], km, cyc_m), 1)
        # cycles already containing each spill row (its level-0 tokens)
        is_spill_row = np.zeros(int(out_idx.max()) + 1, bool)
        is_spill_row[sout] = True
        mm = is_spill_row[out_idx[main_ids]]
        row_cycles = {}
        for r, cc in zip(out_idx[main_ids][mm], cyc_m[mm]):
            row_cycles.setdefault(int(r), set()).add(int(cc))
        for i in range(spill_ids.size):
            bq, kq, rq = int(sb[i]), int(sk[i]), int(sout[i])
            rset = row_cycles.setdefault(rq, set())
            placed = False
            for cc in range(C):
                if fill[bq, kq, cc] < P and cc not in rset:
                    cyc_s[i] = cc
                    gi_s[i] = fill[bq, kq, cc]
                    fill[bq, kq, cc] += 1
                    rset.add(cc)
                    placed = True
                    break
            if not placed:  # overflow: trailing dedicated calls
                cc = C
                while fill[bq, kq, cc] >= P or cc in rset:
                    cc += 1
                cyc_s[i] = cc
                gi_s[i] = fill[bq, kq, cc]
                fill[bq, kq, cc] += 1
                rset.add(cc)
                S = max(S, cc - C + 1)

    CT = C + S  # calls per band
    n_calls = N_BANDS * CT

    ids = np.concatenate([main_ids, spill_ids])
    kk = np.concatenate([km, sk])
    cyc = np.concatenate([cyc_m, cyc_s])
    gi = np.concatenate([gi_m, gi_s])

    s = kk  # slot = k
    fg = s // 4
    lr = s % 4
    bb = band[ids]
    core = bb // N_BANDS
    e = bb % N_BANDS
    call_prog = cyc * N_BANDS + e
    col = call_prog * 256 + fg * P + gi
    p_in_call = s * P + gi

    xq = np.round(x / xscale).astype(np.int8)
    XT = np.zeros((N_CORES, P, n_calls * 256), X_NP)
    for lrv in range(4):
        m = lr == lrv
        XT[:, 32 * lrv : 32 * lrv + 32, :][core[m], :, col[m]] = xq[ids[m]]

    icols = TOK // 16
    IDX = np.full((N_CORES, 16, n_calls * icols), r_band, np.int16)
    IDX[core, p_in_call % 16, call_prog * icols + p_in_call // 16] = rowin_all[ids]

    w = (np.asarray(weight, np.float32) * xscale).astype(ml_dtypes.bfloat16)
    wall = np.zeros((P, 64), ml_dtypes.bfloat16)
    for k in range(8):
        wall[32 * (k % 4) : 32 * (k % 4) + 32, 32 * (k // 4) : 32 * (k // 4) + 32] = w[k]

    cores = [{"xsT": XT[cc], "idx": IDX[cc], "wall": wall} for cc in range(N_CORES)]
    meta = {"r_band": r_band, "r_pad": r_pad, "C": CT, "M": M}
    return cores, meta


def build_bass(meta):
    r_band = meta["r_band"]
    r_pad = meta["r_pad"]
    C = meta["C"]
    n_calls = N_BANDS * C
    FR = r_pad // P  # compaction frames per band
    icols = TOK // 16

    nc = bacc.Bacc("TRN2", num_swdge_queues=N_Q)
    xsT = nc.dram_tensor("xsT", [P, n_calls * 256], X_BIR, kind="ExternalInput")
    idx = nc.dram_tensor("idx", [16, n_calls * icols], mybir.dt.int16, kind="ExternalInput")
    wall = nc.dram_tensor("wall", [P, 64], mybir.dt.bfloat16, kind="ExternalInput")
    yq = nc.dram_tensor("yq", [N_BANDS * r_pad, 32], mybir.dt.int8, kind="ExternalOutput")
    ys = nc.dram_tensor("ys", [N_BANDS * r_pad], mybir.dt.float16, kind="ExternalOutput")
    scr = [
        nc.dram_tensor(f"scr_{q}", [2 * r_pad, 64], mybir.dt.float32, kind="Internal")
        for q in range(N_Q)
    ]
    # block-contiguous per-band views: partition p owns rows [p*FR, (p+1)*FR)
    # of the band -> every DMA span below is contiguous per partition (1
    # descriptor/partition instead of 1 per 128B frame; keeps the NEFF small)
    yqv = [
        yq[e * r_pad : (e + 1) * r_pad, :].rearrange("(p n) c -> p n c", p=P)
        for e in range(N_BANDS)
    ]
    ysv = [
        ys[e * r_pad : (e + 1) * r_pad].rearrange("(p n) -> p n", p=P)
        for e in range(N_BANDS)
    ]

    with tile.TileContext(nc) as tc:
        with (
            tc.tile_pool(name="wp", bufs=1) as wpool,
            tc.tile_pool(name="ip", bufs=1) as ipool,
            tc.tile_pool(name="zp", bufs=1) as zpool,
            tc.tile_pool(name="xp", bufs=4) as xpool,
            tc.tile_pool(name="xc", bufs=4) as xcpool,
            tc.tile_pool(name="st", bufs=8) as stpool,
            tc.tile_pool(name="pz", bufs=8, space="PSUM") as pzpool,
            tc.tile_pool(name="cp", bufs=4) as cpool,
            tc.tile_pool(name="sc", bufs=4) as scpool,
            tc.tile_pool(name="qp", bufs=4) as qpool,
        ):
            wt = wpool.tile([P, 64], mybir.dt.bfloat16, tag="w")
            nc.sync.dma_start(out=wt[:], in_=wall[:, :])
            it = ipool.tile([P, n_calls * icols], mybir.dt.int16, tag="idx")
            for gpart in range(8):
                nc.sync.dma_start(
                    out=it[16 * gpart : 16 * gpart + 16, :], in_=idx[:, :]
                )
            # zero the scatter scratch (block-contiguous per-half views)
            zt = zpool.tile([P, 2048], mybir.dt.float32, tag="z")
            nc.vector.memset(zt[:], 0.0)
            for q in range(N_Q):
                for half in range(2):
                    svh = scr[q][half * r_pad : (half + 1) * r_pad, :].rearrange(
                        "(p n) c -> p n c", p=P
                    )  # [128, FR, 64]
                    f0 = 0
                    while f0 < FR:
                        fcnt = min(32, FR - f0)
                        nc.sync.dma_start(
                            out=svh[:, f0 : f0 + fcnt, :],
                            in_=zt[:, : fcnt * 64].rearrange("p (n c) -> p n c", c=64),
                        )
                        f0 += fcnt

            for c in range(C):
                for e in range(N_BANDS):
                    call_prog = c * N_BANDS + e
                    xb8 = xpool.tile([P, 256], X_BIR, tag="x")
                    nc.sync.dma_start(
                        out=xb8[:], in_=xsT[:, call_prog * 256 : (call_prog + 1) * 256]
                    )
                    xb = xcpool.tile([P, 256], mybir.dt.bfloat16, tag="xc")
                    nc.vector.tensor_copy(out=xb[:], in_=xb8[:])
                    st = stpool.tile([P, 8, 32], mybir.dt.float32, tag="st")
                    for fg in range(2):
                        for lrv in range(4):
                            pz = pzpool.tile([P, 32], mybir.dt.float32, tag="pz")
                            nc.tensor.matmul(
                                out=pz[:],
                                lhsT=xb[32 * lrv : 32 * lrv + 32, P * fg : P * (fg + 1)],
                                rhs=wt[32 * lrv : 32 * lrv + 32, 32 * fg : 32 * fg + 32],
                                start=True,
                                stop=True,
                                tile_position=(32 * lrv, 0),
                            )
                            nc.vector.tensor_copy(out=st[:, 4 * fg + lrv, :], in_=pz[:])
                    q = e % N_Q
                    off = 0 if e < N_Q else r_pad
                    nc.gpsimd.dma_scatter_add(
                        scr[q][off : off + r_band + 1, :32],
                        st[:],
                        it[:, call_prog * icols : (call_prog + 1) * icols],
                        TOK,
                        TOK,
                        32,
                        elem_step=64,
                        queue_num=q,
                    )

            # compact scratch f32 [*, 64] -> int8 rows + fp16 row scales.
            # reads pull the full 64-col row (contiguous per partition); the
            # DVE ops then address only cols :32 via strided APs.
            CH = 16  # frames per chunk
            for e in range(N_BANDS):
                q = e % N_Q
                half = e // N_Q
                svh = scr[q][half * r_pad : (half + 1) * r_pad, :].rearrange(
                    "(p n) c -> p n c", p=P
                )
                f0 = 0
                while f0 < FR:
                    fcnt = min(CH, FR - f0)
                    ct = cpool.tile([P, CH, 64], mybir.dt.float32, tag="ct")
                    nc.sync.dma_start(
                        out=ct[:, :fcnt, :],
                        in_=svh[:, f0 : f0 + fcnt, :],
                    )
                    amax = scpool.tile([P, CH], mybir.dt.float32, tag="amax")
                    nc.vector.tensor_reduce(
                        out=amax[:, :fcnt],
                        in_=ct[:, :fcnt, :32],
                        axis=mybir.AxisListType.X,
                        op=mybir.AluOpType.max,
                        apply_absolute_value=True,
                    )
                    nc.vector.tensor_scalar_max(
                        out=amax[:, :fcnt], in0=amax[:, :fcnt], scalar1=1e-30
                    )
                    sca = scpool.tile([P, CH], mybir.dt.float32, tag="sca")
                    nc.vector.reciprocal(out=sca[:, :fcnt], in_=amax[:, :fcnt])
                    nc.vector.tensor_scalar_mul(
                        out=sca[:, :fcnt], in0=sca[:, :fcnt], scalar1=127.0
                    )
                    smx = scpool.tile([P, CH], mybir.dt.float16, tag="smx")
                    nc.vector.tensor_copy(out=smx[:, :fcnt], in_=amax[:, :fcnt])
                    nc.sync.dma_start(
                        out=ysv[e][:, f0 : f0 + fcnt],
                        in_=smx[:, :fcnt],
                    )
                    nc.vector.tensor_mul(
                        out=ct[:, :fcnt, :32],
                        in0=ct[:, :fcnt, :32],
                        in1=sca[:, :fcnt].unsqueeze(-1).broadcast_to((P, fcnt, 32)),
                    )
                    qt = qpool.tile([P, CH, 32], mybir.dt.int8, tag="qt")
                    nc.vector.tensor_copy(out=qt[:, :fcnt, :], in_=ct[:, :fcnt, :32])
                    nc.sync.dma_start(
                        out=yqv[e][:, f0 : f0 + fcnt, :],
                        in_=qt[:, :fcnt, :],
                    )
                    f0 += fcnt
    nc.compile()
    return nc


def kernel(x, weight, offset_idx, out_idx, num_out):
    from concourse.bass_utils import run_bass_kernel_spmd

    x = np.asarray(x, np.float32)
    weight = np.asarray(weight, np.float32)
    offset_idx = np.asarray(offset_idx, np.int64)
    out_idx = np.asarray(out_idx, np.int64)
    num_out = int(num_out)

    cores, meta = host_prepare(x, weight, offset_idx, out_idx)
    nc = build_bass(meta)
    in_maps = [{"xsT": c["xsT"], "idx": c["idx"], "wall": c["wall"]} for c in cores]
    res = run_bass_kernel_spmd(nc, in_maps, core_ids=list(range(N_CORES)))

    r_band = meta["r_band"]
    r_pad = meta["r_pad"]
    M = meta["M"]
    y = np.zeros((num_out, 32), np.float32)
    for cc in range(N_CORES):
        yqc = res.results[cc]["yq"].reshape(N_BANDS, r_pad, 32)
        ysc = res.results[cc]["ys"].reshape(N_BANDS, r_pad)
        for e in range(N_BANDS):
            gb = cc * N_BANDS + e
            r0 = gb * r_band
            r1 = min(r0 + r_band, M)
            if r1 <= r0:
                continue
            n = r1 - r0
            y[r0:r1] = yqc[e, :n].astype(np.float32) * (
                ysc[e, :n, None].astype(np.float32) / 127.0
            )
    return y
